# revision 18
# baseline (speedup 1.0000x reference)
"""Trainium2 Bass kernel for per-head-projection MultiHeadAttention.

Contract: kernel(**inputs) takes the FULL unsharded inputs (as produced by
reference.setup_inputs()) and returns the FULL [B, S, D] output.

Sharding (tensor-parallel over heads x data-parallel over batch):
  - 8 cores; cores 0-3 handle batch 0, cores 4-7 handle batch 1.
  - Each core owns 4 heads (two "head pairs"): Q/K/V projections for those
    heads, causal attention, and a partial output projection. The host sums
    the 4 bf16 partials per batch and adds bo.

v5 design notes (engines are in-order; overlap = emission interleaving):
  - All matmul operands bf16 (fp32 matmuls are 2-pass half-rate on TRN2).
    PSUM accumulation fp32; output partials bf16.
  - Host-side DMA-friendly layouts: activations [128, NSG, NDC, SG]
    (s-group major, fully contiguous per partition row) and weights
    pre-transposed to [128, NDC, E], so every load streams at full HBM
    bandwidth. g=0 slices load first, across three queues (sync/scalar/
    gpsimd) so the prologue chain is not serialized on one ring.
  - Scores transposed ([keys, queries]), pair row-packed (K=64 tiles at
    partition 0/64); denominator from a ones-column in V-natural; exp is
    one strided [128,2,W] ACTIVATE; diagonal tiles trim to off=tp_i*128.
  - V projected directly into natural [keys, dims] layout (activations
    stationary, both pairs' Wv moving, N=256).
  - sc2 PSUM triple-buffered; the attention inner loop is ACT(exp)-bound
    and all PE-only work (V-nat g+1, output projection g-1, NEXT group's
    Q/K projections, deferred normalize broadcasts) runs as fillers
    inside it.
  - Normalize: ctx PSUM banks are freed immediately after a pair's last
    PV by evicting unnormalized ctx to SBUF (so the next pair's PV is
    never gated by the reciprocal chain); the rank-1 ones-block broadcast
    matmul runs bf16 and is deferred into the next filler stream.
"""

import os
import sys

sys.path.insert(0, "/opt/trn_rl_repo")

import numpy as np

B, S, D, H = 2, 2048, 1024, 16
DH = D // H            # 64
NCORES = 8
HPC = H * B // NCORES  # 4 heads per core
NPAIR = HPC // 2       # 2 head pairs per core
SG = 512               # s-group / query-group size
NSG = S // SG          # 4
NKT = S // 128         # 16 key tiles
NDC = D // 128         # 8 contraction chunks

_BUILD_CACHE = {}


def _build(causal: bool, bz: bool, debug_dumps: bool = False):
    """Build + compile the per-core Bass program.

    bz: all of bq/bk/bv are zero -> skip bias adds (plain casts instead).
    """
    import concourse.bass as bass
    import concourse.bacc as bacc
    import concourse.tile as tile
    from concourse import mybir

    f32 = mybir.dt.float32
    bf16 = mybir.dt.bfloat16
    EXP = mybir.ActivationFunctionType.Exp

    nc = bacc.Bacc("TRN2", target_bir_lowering=False, debug=False)

    # activations: [partition, s-group, chunk, s] -- contiguous per row
    xq = nc.dram_tensor("xq", [128, NSG, NDC, SG], bf16, kind="ExternalInput").ap()
    xk = nc.dram_tensor("xk", [128, NSG, NDC, SG], bf16, kind="ExternalInput").ap()
    xv = nc.dram_tensor("xv", [128, NSG, NDC, SG], bf16, kind="ExternalInput").ap()
    # weights pre-transposed to SBUF layout
    wq = nc.dram_tensor("wq", [NPAIR, 128, NDC, 128], bf16, kind="ExternalInput").ap()
    wk = nc.dram_tensor("wk", [NPAIR, 128, NDC, 128], bf16, kind="ExternalInput").ap()
    wv = nc.dram_tensor("wv", [128, NDC, 256], bf16, kind="ExternalInput").ap()
    wo = nc.dram_tensor("wo", [NPAIR, 128, D], bf16, kind="ExternalInput").ap()
    mk = nc.dram_tensor("mk", [128, 128], bf16, kind="ExternalInput").ap()
    on = nc.dram_tensor("on", [128, 64], bf16, kind="ExternalInput").ap()
    bqk = nc.dram_tensor("bqk", [2, NPAIR, 128, 1], f32, kind="ExternalInput").ap()
    bvr = nc.dram_tensor("bvr", [1, NPAIR * 128], bf16, kind="ExternalInput").ap()
    on1 = nc.dram_tensor("on1", [1, 128], bf16, kind="ExternalInput").ap()
    obk = nc.dram_tensor("obk", [33, 128], bf16, kind="ExternalInput").ap()
    out = nc.dram_tensor("out", [S, D], bf16, kind="ExternalOutput").ap()
    if debug_dumps:
        d_qT = nc.dram_tensor("d_qT", [128, NPAIR, S], bf16, kind="ExternalOutput").ap()
        d_kT = nc.dram_tensor("d_kT", [128, NPAIR, S], bf16, kind="ExternalOutput").ap()
        d_vN = nc.dram_tensor(
            "d_vN", [128, NPAIR, NKT, 2, 65], bf16, kind="ExternalOutput"
        ).ap()
        d_ctxn = nc.dram_tensor(
            "d_ctxn", [128, NPAIR, S], bf16, kind="ExternalOutput"
        ).ap()

    with tile.TileContext(nc) as tc:
        with (
            tc.tile_pool(name="persist", bufs=1) as persist,
            tc.tile_pool(name="pts", bufs=4) as pt_pool,
            tc.tile_pool(name="outs", bufs=6) as out_pool,
            tc.tile_pool(name="rbs", bufs=2) as rb_pool,
            tc.tile_pool(name="ctxus", bufs=2) as cu_pool,
            # psA (3 x 2 banks) serves pp/sc2/vnat/rb/op; psB (2 x 1 bank)
            # holds one pair's ctx accumulators at a time.
            tc.tile_pool(name="psma", bufs=3, space="PSUM") as psA,
            tc.tile_pool(name="psmb", bufs=2, space="PSUM") as psB,
        ):
            # ---- startup loads. Bulk data rides the two fast HWDGE rings
            # (sync: wq + per-group xq/xv; scalar: wk/wv + per-group xk); the
            # slow gpsimd SWDGE queue gets only small constants + wo.
            # Per-(tensor, group) tiles give fine-grained dependencies.
            x_sb = {}
            for t_i in range(3):
                for g in range(NSG):
                    x_sb[t_i, g] = persist.tile(
                        [128, NDC, SG], bf16, tag=f"x{t_i}{g}", name=f"x{t_i}{g}"
                    )
            w_sb = {}
            for t_i, wd in enumerate([wq, wk]):
                for p in range(NPAIR):
                    w_sb[t_i, p] = persist.tile(
                        [128, NDC, 128], bf16, tag=f"w{t_i}{p}", name=f"w{t_i}{p}"
                    )
            wv_sb = persist.tile([128, NDC, 256], bf16, tag="wv")

            # PE warmup: throwaway matmuls on a memset tile keep the HAM
            # clock gate open across prologue DMA waits.
            wrm = persist.tile([128, SG], bf16, tag="wrm")
            nc.vector.memset(wrm, 0.5)

            def warmup(n):
                for _ in range(n):
                    wps = psA.tile([128, SG], f32, tag="big", name="wps")
                    nc.tensor.matmul(
                        wps, lhsT=wrm[:, 0:128], rhs=wrm, start=True, stop=True
                    )

            warmup(24)

            nc.sync.dma_start(out=w_sb[0, 0], in_=wq[0])
            nc.sync.dma_start(out=w_sb[0, 1], in_=wq[1])
            nc.scalar.dma_start(out=w_sb[1, 0], in_=wk[0])
            nc.scalar.dma_start(out=w_sb[1, 1], in_=wk[1])
            nc.sync.dma_start(out=x_sb[0, 0][:, 0:4, :], in_=xq[:, 0, 0:4])
            nc.sync.dma_start(out=x_sb[0, 0][:, 4:8, :], in_=xq[:, 0, 4:8])
            nc.scalar.dma_start(out=x_sb[1, 0][:, 0:4, :], in_=xk[:, 0, 0:4])
            nc.scalar.dma_start(out=x_sb[1, 0][:, 4:8, :], in_=xk[:, 0, 4:8])
            nc.scalar.dma_start(out=wv_sb, in_=wv)
            nc.sync.dma_start(out=x_sb[2, 0][:, 0:4, :], in_=xv[:, 0, 0:4])
            nc.scalar.dma_start(out=x_sb[2, 0][:, 4:8, :], in_=xv[:, 0, 4:8])
            for g in range(1, NSG):
                nc.sync.dma_start(out=x_sb[0, g], in_=xq[:, g])
                nc.sync.dma_start(out=x_sb[2, g], in_=xv[:, g])
                nc.scalar.dma_start(out=x_sb[1, g], in_=xk[:, g])

            mask = persist.tile([128, 128], bf16, tag="mask")
            nc.gpsimd.dma_start(out=mask, in_=mk)
            obk_sb = persist.tile([33, 128], bf16, tag="obk")
            nc.gpsimd.dma_start(out=obk_sb, in_=obk)

            qT = persist.tile([128, NPAIR, S], bf16, tag="qT")
            kT = persist.tile([128, NPAIR, S], bf16, tag="kT")
            vN = persist.tile([128, NPAIR, NKT, 2, 65], bf16, tag="vN")
            ctxn = persist.tile([128, NPAIR, S], bf16, tag="ctxn")

            # ones columns of V-natural (softmax denominator trick)
            nc.gpsimd.dma_start(
                out=vN[:, :, :, :, 64],
                in_=on.rearrange("q (p k h) -> q p k h", p=NPAIR, k=NKT),
            )
            wo_sb = persist.tile([128, NPAIR, D], bf16, tag="wo")
            for p in range(NPAIR):
                nc.gpsimd.dma_start(out=wo_sb[:, p, :], in_=wo[p])
            if not bz:
                b_sb = persist.tile([128, 2, NPAIR, 1], f32, tag="b")
                nc.gpsimd.dma_start(out=b_sb, in_=bqk.rearrange("t p q o -> q t p o"))
                bv_sb = persist.tile([1, 256], bf16, tag="bv")
                nc.gpsimd.dma_start(out=bv_sb, in_=bvr)
                on1_sb = persist.tile([1, 128], bf16, tag="on1")
                nc.gpsimd.dma_start(out=on1_sb, in_=on1)

            # fp32 staging rows for the denominators (rows 0/32 hold the two
            # heads' values; the rest memset once so the reciprocal and the
            # bf16 cast never see garbage), ping-ponged per pair.
            stages = []
            for i in range(2):
                stg = persist.tile([33, SG], f32, tag=f"stg{i}", name=f"stg{i}")
                stgr = persist.tile([33, SG], f32, tag=f"sgr{i}", name=f"sgr{i}")
                stgb = persist.tile([33, SG], bf16, tag=f"sgb{i}", name=f"sgb{i}")
                nc.vector.memset(stg, 1.0)
                stages.append((stg, stgr, stgb))

            out_q = [0]

            def emit_qk_proj_units(t_i, g):
                """8 filler closures (one per chunk) computing qT/kT for g."""
                st = {}

                def unit(c, t_i=t_i, g=g, st=st):
                    if c == 0:
                        st["pp"] = psA.tile([128, 2, SG], f32, tag="big", name="pp")
                    pp = st["pp"]
                    for p in range(NPAIR):
                        nc.tensor.matmul(
                            pp[:, p, :],
                            lhsT=w_sb[t_i, p][:, c, :],
                            rhs=x_sb[t_i, g][:, c, :],
                            start=(c == 0),
                            stop=(c == NDC - 1),
                        )
                    if c == NDC - 1:
                        gs = slice(g * SG, (g + 1) * SG)
                        dstT = qT if t_i == 0 else kT
                        for p in range(NPAIR):
                            if bz:
                                nc.vector.tensor_copy(dstT[:, p, gs], pp[:, p, :])
                            else:
                                nc.vector.tensor_scalar_add(
                                    out=dstT[:, p, gs],
                                    in0=pp[:, p, :],
                                    scalar1=b_sb[:, t_i, p, :],
                                )

                return [
                    (lambda c=c: (unit(c), unit(c + 1)))
                    for c in range(0, NDC, 2)
                ]

            def emit_vnat_unit(kt):
                # V-natural projection for one key-tile, both pairs at once:
                # activations stationary, both pairs' Wv moving (N=256).
                g = kt // 4
                k4 = kt % 4
                vp = psA.tile([128, 2, 2, 64], f32, tag="big", name="vp")
                for c in range(NDC):
                    nc.tensor.matmul(
                        vp,
                        lhsT=x_sb[2, g][:, c, k4 * 128 : (k4 + 1) * 128],
                        rhs=wv_sb[:, c, :],
                        start=(c == 0),
                        stop=(bz and c == NDC - 1),
                    )
                if not bz:
                    nc.tensor.matmul(
                        vp, lhsT=on1_sb, rhs=bv_sb, start=False, stop=True
                    )
                for p in range(NPAIR):
                    nc.vector.tensor_copy(
                        out=vN[:, p, kt, :, 0:64], in_=vp[:, p, :, :]
                    )

            def emit_outproj_unit(g, st4, n, tail=False):
                # one [128s x 512n] tile of the partial output projection
                srow = (4 * g + st4) * 128
                op = psA.tile([128, SG], f32, tag="big", name="op")
                for p in range(NPAIR):
                    nc.tensor.matmul(
                        op,
                        lhsT=ctxn[:, p, srow : srow + 128],
                        rhs=wo_sb[:, p, n * SG : (n + 1) * SG],
                        start=(p == 0),
                        stop=(p == NPAIR - 1),
                    )
                ob = out_pool.tile([128, SG], bf16, tag="ob", name="ob")
                if tail and out_q[0] % 2 == 0:
                    nc.scalar.copy(ob, op)
                else:
                    nc.vector.tensor_copy(ob, op)
                out_q[0] += 1
                nc.sync.dma_start(
                    out=out[srow : srow + 128, n * SG : (n + 1) * SG], in_=ob
                )

            def emit_attention(g, fillers):
                """Attention for q-group g, draining `fillers` into the
                ACT-bound inner loop. Returns the deferred normalize-finish
                closure of the last pair."""
                gs = slice(g * SG, (g + 1) * SG)
                nkc = (4 * g + 4) if causal else NKT
                slots = [2 * nkc]
                fin = None
                for p in range(NPAIR):
                    ctx2 = [
                        psB.tile([65, SG], f32, tag="ctx", name="ctx")
                        for _ in range(2)
                    ]

                    def emit_pv(kc, off, pt2, ctx2=ctx2, p=p, nkc=nkc):
                        for h_s in range(2):
                            nc.tensor.matmul(
                                ctx2[h_s][:, off:SG],
                                lhsT=vN[:, p, kc, h_s, :],
                                rhs=pt2[:, h_s, off:],
                                start=(kc == 0),
                                stop=(kc == nkc - 1),
                            )

                    # software-pipelined: sc/exp(kc) emitted before pv(kc-1)
                    prev = None
                    for kc in range(nkc):
                        tp_i = kc - 4 * g
                        diag = causal and tp_i >= 0
                        off = tp_i * 128 if diag else 0
                        sc2 = psA.tile([128, 2, SG], f32, tag="big", name="sc2")
                        for h_s in range(2):
                            hp = slice(h_s * 64, (h_s + 1) * 64)
                            nc.tensor.matmul(
                                sc2[:, h_s, off:],
                                lhsT=kT[hp, p, kc * 128 : (kc + 1) * 128],
                                rhs=qT[hp, p, g * SG + off : (g + 1) * SG],
                                start=True,
                                stop=True,
                            )
                        pt2 = pt_pool.tile([128, 2, SG], bf16, tag="pt", name="pt2")
                        nc.scalar.activation(
                            pt2[:, :, off:], sc2[:, :, off:], EXP, scale=0.125
                        )
                        if diag:
                            d0 = tp_i * 128
                            for h_s in range(2):
                                nc.vector.tensor_mul(
                                    pt2[:, h_s, d0 : d0 + 128],
                                    pt2[:, h_s, d0 : d0 + 128],
                                    mask,
                                )
                        # drain fillers: keep pace with remaining slots
                        pops = 1
                        if len(fillers) > slots[0] > 0:
                            pops = min(2, -(-len(fillers) // slots[0]))
                        for _ in range(pops):
                            if fillers:
                                fillers.pop(0)()
                        slots[0] -= 1
                        if prev is not None:
                            emit_pv(*prev)
                        prev = (kc, off, pt2)
                    emit_pv(*prev)

                    # ---- normalize phase 1: evict unnormalized ctx to SBUF
                    # (frees the psB banks; scalar engine does the PSUM-shift
                    # copies) and stage denominators at rows 0/32 for a
                    # lane-aligned reciprocal. Deferred into the NEXT pair's
                    # filler stream so it never sits between two pairs' exps
                    # in the ACT queue.
                    ctxu = cu_pool.tile([128, SG], f32, tag="cu", name="cu")
                    stg, stgr, stgb = stages[p]

                    def ph1_fn(p=p, ctx2=ctx2, ctxu=ctxu, stg=stg, stgr=stgr,
                               stgb=stgb):
                        for h_s in range(2):
                            nc.vector.tensor_copy(
                                ctxu[h_s * 64 : (h_s + 1) * 64, :],
                                ctx2[h_s][0:64, :],
                            )
                            nc.vector.tensor_copy(
                                stg[32 * h_s : 32 * h_s + 1, :],
                                ctx2[h_s][64:65, :],
                            )
                        nc.vector.reciprocal_approx_fast(out=stgr, in_=stg)
                        nc.vector.tensor_copy(stgb, stgr)

                    def fin_fn(p=p, stgb=stgb, ctxu=ctxu):
                        # rank-1 broadcast of 1/den (PE) + per-head multiply
                        rb = psA.tile([128, SG], f32, tag="big", name="rb")
                        nc.tensor.matmul(
                            rb, lhsT=obk_sb, rhs=stgb, start=True, stop=True
                        )
                        rbs = rb_pool.tile([128, SG], f32, tag="rbs", name="rbs")
                        nc.vector.tensor_copy(rbs, rb)
                        for h_s in range(2):
                            hp = slice(h_s * 64, (h_s + 1) * 64)
                            nc.vector.tensor_mul(
                                ctxn[hp, p, gs], ctxu[hp, :], rbs[hp, :]
                            )

                    if p == 0:
                        fillers.insert(0, ph1_fn)
                        fillers.insert(min(3, len(fillers)), fin_fn)
                    else:
                        fin = (ph1_fn, fin_fn)
                # drain leftover fillers
                while fillers:
                    fillers.pop(0)()
                return fin

            def outproj_fillers(g):
                return [
                    (lambda g=g, st4=st4, n=n: emit_outproj_unit(g, st4, n))
                    for st4 in range(4)
                    for n in range(D // SG)
                ]

            if causal:
                # prologue: Q/K proj for g0, first V-nat tile; the rest of
                # g0's V-nat units lead the g0 filler stream.
                for u in emit_qk_proj_units(0, 0):
                    u()
                for u in emit_qk_proj_units(1, 0):
                    u()
                emit_vnat_unit(0)
                fin_prev = None
                for g in range(NSG):
                    fillers = []
                    if g == 0:
                        fillers += [lambda kt=kt: emit_vnat_unit(kt) for kt in (1, 2, 3)]
                    if fin_prev is not None:
                        fillers.append(fin_prev[0])
                        fillers.append(fin_prev[1])
                    if g > 0:
                        ops = outproj_fillers(g - 1)
                        vts = [
                            (lambda kt=kt: emit_vnat_unit(kt))
                            for kt in range(4 * g + 4, 4 * g + 8)
                        ] if g + 1 < NSG else []
                        mixed = []
                        for i, f in enumerate(ops):
                            mixed.append(f)
                            if i % 2 == 1 and vts:
                                mixed.append(vts.pop(0))
                        fillers += mixed + vts
                    elif g + 1 < NSG:
                        fillers += [
                            (lambda kt=kt: emit_vnat_unit(kt))
                            for kt in range(4 * g + 4, 4 * g + 8)
                        ]
                    if g + 1 < NSG:
                        fillers += emit_qk_proj_units(0, g + 1)
                        fillers += emit_qk_proj_units(1, g + 1)
                    fin_prev = emit_attention(g, fillers)
                fin_prev[0]()
                fin_prev[1]()
                for st4 in range(4):
                    for n in range(D // SG):
                        emit_outproj_unit(NSG - 1, st4, n, tail=True)
                if debug_dumps:
                    nc.sync.dma_start(out=d_qT, in_=qT[:, :, :])
                    nc.sync.dma_start(out=d_kT, in_=kT[:, :, :])
                    nc.sync.dma_start(out=d_vN, in_=vN[:, :, :, :, :])
                    nc.sync.dma_start(out=d_ctxn, in_=ctxn[:, :, :])
            else:
                # non-causal: attention needs ALL key tiles -> run every
                # projection first, then attention with outproj fillers.
                for g in range(NSG):
                    for u in emit_qk_proj_units(0, g):
                        u()
                    for u in emit_qk_proj_units(1, g):
                        u()
                    for kt in range(4 * g, 4 * g + 4):
                        emit_vnat_unit(kt)
                fin_prev = None
                for g in range(NSG):
                    fillers = []
                    if fin_prev is not None:
                        fillers.append(fin_prev[0])
                        fillers.append(fin_prev[1])
                    if g > 0:
                        fillers += outproj_fillers(g - 1)
                    fin_prev = emit_attention(g, fillers)
                fin_prev[0]()
                fin_prev[1]()
                for st4 in range(4):
                    for n in range(D // SG):
                        emit_outproj_unit(NSG - 1, st4, n, tail=True)

    nc.compile()
    return nc


def _core_inputs(query, key, value, Wq, bq, Wk, bk, Wv, bv, Wo, core):
    import ml_dtypes

    bf16 = ml_dtypes.bfloat16
    b = core // (NCORES // B)
    h0 = (core % (NCORES // B)) * HPC
    f32 = np.float32

    def packx(x):
        # [S, D] -> [128, NSG, NDC, SG]: xT[c*128+q, g*SG+s] at [q, g, c, s]
        xt = np.asarray(x, f32).T.reshape(NDC, 128, NSG, SG)
        return np.ascontiguousarray(xt.transpose(1, 2, 0, 3).astype(bf16))

    def packw(W):
        # [H, D, DH] -> per-pair [128, NDC, 128]
        outw = []
        for p in range(NPAIR):
            wp = np.concatenate([W[h0 + 2 * p], W[h0 + 2 * p + 1]], axis=1)  # [D,128]
            outw.append(wp.reshape(NDC, 128, 128).transpose(1, 0, 2))
        return np.ascontiguousarray(np.stack(outw).astype(bf16))

    def packb(bias):
        return np.ascontiguousarray(
            np.stack(
                [
                    np.concatenate([bias[h0 + 2 * p], bias[h0 + 2 * p + 1]])
                    for p in range(NPAIR)
                ]
            ).reshape(NPAIR, 128, 1),
            dtype=f32,
        )

    wo_p = np.ascontiguousarray(
        np.stack(
            [Wo[(h0 + 2 * p) * DH : (h0 + 2 * p + 2) * DH] for p in range(NPAIR)]
        ).astype(bf16)
    )
    wv_p = np.concatenate(
        [
            np.concatenate([Wv[h0 + 2 * p], Wv[h0 + 2 * p + 1]], axis=1)
            for p in range(NPAIR)
        ],
        axis=1,
    )  # [D, 256]
    wv_p = np.ascontiguousarray(
        wv_p.reshape(NDC, 128, 256).transpose(1, 0, 2).astype(bf16)
    )
    kk, qq = np.meshgrid(np.arange(128), np.arange(128), indexing="ij")
    mkk = (kk <= qq).astype(bf16)  # key <= query (keys on partitions)
    obk = np.zeros((33, 128), bf16)
    obk[0, 0:64] = 1.0
    obk[32, 64:128] = 1.0
    bvr = np.concatenate(
        [
            np.concatenate([bv[h0 + 2 * p], bv[h0 + 2 * p + 1]])
            for p in range(NPAIR)
        ]
    ).reshape(1, 256).astype(bf16)
    return {
        "mk": mkk,
        "on": np.ones((128, 64), bf16),
        "on1": np.ones((1, 128), bf16),
        "obk": obk,
        "xq": packx(query[b]),
        "xk": packx(key[b]),
        "xv": packx(value[b]),
        "wq": packw(Wq),
        "wk": packw(Wk),
        "wv": wv_p,
        "wo": wo_p,
        "bqk": np.stack([packb(bq), packb(bk)]),
        "bvr": bvr,
    }


LAST_RESULTS = None


def kernel(query, key, value, Wq, bq, Wk, bk, Wv, bv, Wo, bo, look_ahead_mask):
    global LAST_RESULTS
    from concourse.bass_utils import run_bass_kernel_spmd

    query = np.asarray(query, dtype=np.float32)
    key = np.asarray(key, dtype=np.float32)
    value = np.asarray(value, dtype=np.float32)
    Wq, Wk, Wv = (np.asarray(a, dtype=np.float32) for a in (Wq, Wk, Wv))
    bq, bk, bv = (np.asarray(a, dtype=np.float32) for a in (bq, bk, bv))
    Wo = np.asarray(Wo, dtype=np.float32)
    bo = np.asarray(bo, dtype=np.float32)
    causal = bool(np.asarray(look_ahead_mask).item())
    bz = not (np.any(bq) or np.any(bk) or np.any(bv))

    if (causal, bz) not in _BUILD_CACHE:
        _BUILD_CACHE[causal, bz] = _build(causal, bz)
    nc = _BUILD_CACHE[causal, bz]

    in_maps = [
        _core_inputs(query, key, value, Wq, bq, Wk, bk, Wv, bv, Wo, c)
        for c in range(NCORES)
    ]
    res = run_bass_kernel_spmd(nc, in_maps, core_ids=list(range(NCORES)))
    LAST_RESULTS = res

    gpb = NCORES // B
    out = np.stack(
        [
            np.sum(
                [
                    res.results[b * gpb + i]["out"].astype(np.float32)
                    for i in range(gpb)
                ],
                axis=0,
            )
            for b in range(B)
        ]
    )
    return (out + bo[None, None, :]).astype(np.float32)


# revision 20
# speedup vs baseline: 1.0255x; 1.0255x over previous
"""Trainium2 Bass kernel for per-head-projection MultiHeadAttention.

Contract: kernel(**inputs) takes the FULL unsharded inputs (as produced by
reference.setup_inputs()) and returns the FULL [B, S, D] output.

Sharding (tensor-parallel over heads x data-parallel over batch):
  - 8 cores; cores 0-3 handle batch 0, cores 4-7 handle batch 1.
  - Each core owns 4 heads (two "head pairs"): Q/K/V projections for those
    heads, causal attention, and a partial output projection. The host sums
    the 4 bf16 partials per batch and adds bo.

v5 design notes (engines are in-order; overlap = emission interleaving):
  - All matmul operands bf16 (fp32 matmuls are 2-pass half-rate on TRN2).
    PSUM accumulation fp32; output partials bf16.
  - Host-side DMA-friendly layouts: activations [128, NSG, NDC, SG]
    (s-group major, fully contiguous per partition row) and weights
    pre-transposed to [128, NDC, E], so every load streams at full HBM
    bandwidth. g=0 slices load first, across three queues (sync/scalar/
    gpsimd) so the prologue chain is not serialized on one ring.
  - Scores transposed ([keys, queries]), pair row-packed (K=64 tiles at
    partition 0/64); denominator from a ones-column in V-natural; exp is
    one strided [128,2,W] ACTIVATE; diagonal tiles trim to off=tp_i*128.
  - V projected directly into natural [keys, dims] layout (activations
    stationary, both pairs' Wv moving, N=256).
  - sc2 PSUM triple-buffered; the attention inner loop is ACT(exp)-bound
    and all PE-only work (V-nat g+1, output projection g-1, NEXT group's
    Q/K projections, deferred normalize broadcasts) runs as fillers
    inside it.
  - Normalize: ctx PSUM banks are freed immediately after a pair's last
    PV by evicting unnormalized ctx to SBUF (so the next pair's PV is
    never gated by the reciprocal chain); the rank-1 ones-block broadcast
    matmul runs bf16 and is deferred into the next filler stream.
"""

import os
import sys

sys.path.insert(0, "/opt/trn_rl_repo")

import numpy as np

B, S, D, H = 2, 2048, 1024, 16
DH = D // H            # 64
NCORES = 8
HPC = H * B // NCORES  # 4 heads per core
NPAIR = HPC // 2       # 2 head pairs per core
SG = 512               # s-group / query-group size
NSG = S // SG          # 4
NKT = S // 128         # 16 key tiles
NDC = D // 128         # 8 contraction chunks

_BUILD_CACHE = {}


def _build(causal: bool, bz: bool, debug_dumps: bool = False):
    """Build + compile the per-core Bass program.

    bz: all of bq/bk/bv are zero -> skip bias adds (plain casts instead).
    """
    import concourse.bass as bass
    import concourse.bacc as bacc
    import concourse.tile as tile
    from concourse import mybir

    f32 = mybir.dt.float32
    bf16 = mybir.dt.bfloat16
    EXP = mybir.ActivationFunctionType.Exp

    nc = bacc.Bacc("TRN2", target_bir_lowering=False, debug=False)

    # activations: [partition, s-group, chunk, s] -- contiguous per row
    xq = nc.dram_tensor("xq", [128, NSG, NDC, SG], bf16, kind="ExternalInput").ap()
    xk = nc.dram_tensor("xk", [128, NSG, NDC, SG], bf16, kind="ExternalInput").ap()
    xv = nc.dram_tensor("xv", [128, NSG, NDC, SG], bf16, kind="ExternalInput").ap()
    # weights pre-transposed to SBUF layout
    wq = nc.dram_tensor("wq", [NPAIR, 128, NDC, 128], bf16, kind="ExternalInput").ap()
    wk = nc.dram_tensor("wk", [NPAIR, 128, NDC, 128], bf16, kind="ExternalInput").ap()
    wv = nc.dram_tensor("wv", [128, NDC, 256], bf16, kind="ExternalInput").ap()
    wo = nc.dram_tensor("wo", [NPAIR, 128, D], bf16, kind="ExternalInput").ap()
    mk = nc.dram_tensor("mk", [128, 128], bf16, kind="ExternalInput").ap()
    on = nc.dram_tensor("on", [128, 64], bf16, kind="ExternalInput").ap()
    bqk = nc.dram_tensor("bqk", [2, NPAIR, 128, 1], f32, kind="ExternalInput").ap()
    bvr = nc.dram_tensor("bvr", [1, NPAIR * 128], bf16, kind="ExternalInput").ap()
    on1 = nc.dram_tensor("on1", [1, 128], bf16, kind="ExternalInput").ap()
    obk = nc.dram_tensor("obk", [33, 128], bf16, kind="ExternalInput").ap()
    out = nc.dram_tensor("out", [S, D], bf16, kind="ExternalOutput").ap()
    if debug_dumps:
        d_qT = nc.dram_tensor("d_qT", [128, NPAIR, S], bf16, kind="ExternalOutput").ap()
        d_kT = nc.dram_tensor("d_kT", [128, NPAIR, S], bf16, kind="ExternalOutput").ap()
        d_vN = nc.dram_tensor(
            "d_vN", [128, NPAIR, NKT, 2, 65], bf16, kind="ExternalOutput"
        ).ap()
        d_ctxn = nc.dram_tensor(
            "d_ctxn", [128, NPAIR, S], bf16, kind="ExternalOutput"
        ).ap()

    with tile.TileContext(nc) as tc:
        with (
            tc.tile_pool(name="persist", bufs=1) as persist,
            tc.tile_pool(name="pts", bufs=4) as pt_pool,
            tc.tile_pool(name="outs", bufs=6) as out_pool,
            tc.tile_pool(name="rbs", bufs=2) as rb_pool,
            tc.tile_pool(name="ctxus", bufs=2) as cu_pool,
            # psA (3 x 2 banks) serves pp/sc2/vnat/rb/op; psB (2 x 1 bank)
            # holds one pair's ctx accumulators at a time.
            tc.tile_pool(name="psma", bufs=3, space="PSUM") as psA,
            tc.tile_pool(name="psmb", bufs=2, space="PSUM") as psB,
        ):
            # ---- startup loads. Bulk data rides the two fast HWDGE rings
            # (sync: wq + per-group xq/xv; scalar: wk/wv + per-group xk); the
            # slow gpsimd SWDGE queue gets only small constants + wo.
            # Per-(tensor, group) tiles give fine-grained dependencies.
            x_sb = {}
            for t_i in range(3):
                for g in range(NSG):
                    x_sb[t_i, g] = persist.tile(
                        [128, NDC, SG], bf16, tag=f"x{t_i}{g}", name=f"x{t_i}{g}"
                    )
            w_sb = {}
            for t_i, wd in enumerate([wq, wk]):
                for p in range(NPAIR):
                    w_sb[t_i, p] = persist.tile(
                        [128, NDC, 128], bf16, tag=f"w{t_i}{p}", name=f"w{t_i}{p}"
                    )
            wv_sb = persist.tile([128, NDC, 256], bf16, tag="wv")

            # PE warmup: throwaway matmuls on a memset tile keep the HAM
            # clock gate open across prologue DMA waits.
            wrm = persist.tile([128, SG], bf16, tag="wrm")
            nc.vector.memset(wrm, 0.5)

            def warmup(n):
                for _ in range(n):
                    wps = psA.tile([128, SG], f32, tag="big", name="wps")
                    nc.tensor.matmul(
                        wps, lhsT=wrm[:, 0:128], rhs=wrm, start=True, stop=True
                    )

            warmup(44)

            nc.sync.dma_start(out=w_sb[0, 0], in_=wq[0])
            nc.sync.dma_start(out=w_sb[0, 1], in_=wq[1])
            nc.scalar.dma_start(out=w_sb[1, 0], in_=wk[0])
            nc.scalar.dma_start(out=w_sb[1, 1], in_=wk[1])
            nc.sync.dma_start(out=x_sb[0, 0][:, 0:4, :], in_=xq[:, 0, 0:4])
            nc.sync.dma_start(out=x_sb[0, 0][:, 4:8, :], in_=xq[:, 0, 4:8])
            nc.scalar.dma_start(out=x_sb[1, 0][:, 0:4, :], in_=xk[:, 0, 0:4])
            nc.scalar.dma_start(out=x_sb[1, 0][:, 4:8, :], in_=xk[:, 0, 4:8])
            nc.scalar.dma_start(out=wv_sb, in_=wv)
            nc.sync.dma_start(out=x_sb[2, 0][:, 0:4, :], in_=xv[:, 0, 0:4])
            nc.scalar.dma_start(out=x_sb[2, 0][:, 4:8, :], in_=xv[:, 0, 4:8])
            for g in range(1, NSG):
                nc.sync.dma_start(out=x_sb[0, g], in_=xq[:, g])
                nc.sync.dma_start(out=x_sb[2, g], in_=xv[:, g])
                nc.scalar.dma_start(out=x_sb[1, g], in_=xk[:, g])

            mask = persist.tile([128, 128], bf16, tag="mask")
            nc.gpsimd.dma_start(out=mask, in_=mk)
            obk_sb = persist.tile([33, 128], bf16, tag="obk")
            nc.gpsimd.dma_start(out=obk_sb, in_=obk)

            qT = persist.tile([128, NPAIR, S], bf16, tag="qT")
            kT = persist.tile([128, NPAIR, S], bf16, tag="kT")
            vN = persist.tile([128, NPAIR, NKT, 2, 65], bf16, tag="vN")
            ctxn = persist.tile([128, NPAIR, S], bf16, tag="ctxn")

            # ones columns of V-natural (softmax denominator trick)
            nc.gpsimd.dma_start(
                out=vN[:, :, :, :, 64],
                in_=on.rearrange("q (p k h) -> q p k h", p=NPAIR, k=NKT),
            )
            wo_sb = persist.tile([128, NPAIR, D], bf16, tag="wo")
            for p in range(NPAIR):
                nc.gpsimd.dma_start(out=wo_sb[:, p, :], in_=wo[p])
            if not bz:
                b_sb = persist.tile([128, 2, NPAIR, 1], f32, tag="b")
                nc.gpsimd.dma_start(out=b_sb, in_=bqk.rearrange("t p q o -> q t p o"))
                bv_sb = persist.tile([1, 256], bf16, tag="bv")
                nc.gpsimd.dma_start(out=bv_sb, in_=bvr)
                on1_sb = persist.tile([1, 128], bf16, tag="on1")
                nc.gpsimd.dma_start(out=on1_sb, in_=on1)

            # fp32 staging rows for the denominators (rows 0/32 hold the two
            # heads' values; the rest memset once so the reciprocal and the
            # bf16 cast never see garbage), ping-ponged per pair.
            stages = []
            for i in range(2):
                stg = persist.tile([33, SG], f32, tag=f"stg{i}", name=f"stg{i}")
                stgr = persist.tile([33, SG], f32, tag=f"sgr{i}", name=f"sgr{i}")
                stgb = persist.tile([33, SG], bf16, tag=f"sgb{i}", name=f"sgb{i}")
                nc.vector.memset(stg, 1.0)
                stages.append((stg, stgr, stgb))

            out_q = [0]

            def emit_qk_proj_units(t_i, g):
                """8 filler closures (one per chunk) computing qT/kT for g."""
                st = {}

                def unit(c, t_i=t_i, g=g, st=st):
                    if c == 0:
                        st["pp"] = psA.tile([128, 2, SG], f32, tag="big", name="pp")
                    pp = st["pp"]
                    for p in range(NPAIR):
                        nc.tensor.matmul(
                            pp[:, p, :],
                            lhsT=w_sb[t_i, p][:, c, :],
                            rhs=x_sb[t_i, g][:, c, :],
                            start=(c == 0),
                            stop=(c == NDC - 1),
                        )
                    if c == NDC - 1:
                        gs = slice(g * SG, (g + 1) * SG)
                        dstT = qT if t_i == 0 else kT
                        for p in range(NPAIR):
                            if bz:
                                nc.vector.tensor_copy(dstT[:, p, gs], pp[:, p, :])
                            else:
                                nc.vector.tensor_scalar_add(
                                    out=dstT[:, p, gs],
                                    in0=pp[:, p, :],
                                    scalar1=b_sb[:, t_i, p, :],
                                )

                return [
                    (lambda c=c: (unit(c), unit(c + 1)))
                    for c in range(0, NDC, 2)
                ]

            def emit_vnat_unit(kt):
                # V-natural projection for one key-tile, both pairs at once:
                # activations stationary, both pairs' Wv moving (N=256).
                g = kt // 4
                k4 = kt % 4
                vp = psA.tile([128, 2, 2, 64], f32, tag="big", name="vp")
                for c in range(NDC):
                    nc.tensor.matmul(
                        vp,
                        lhsT=x_sb[2, g][:, c, k4 * 128 : (k4 + 1) * 128],
                        rhs=wv_sb[:, c, :],
                        start=(c == 0),
                        stop=(bz and c == NDC - 1),
                    )
                if not bz:
                    nc.tensor.matmul(
                        vp, lhsT=on1_sb, rhs=bv_sb, start=False, stop=True
                    )
                for p in range(NPAIR):
                    nc.vector.tensor_copy(
                        out=vN[:, p, kt, :, 0:64], in_=vp[:, p, :, :]
                    )

            def emit_outproj_unit(g, st4, n, tail=False):
                # one [128s x 512n] tile of the partial output projection
                srow = (4 * g + st4) * 128
                op = psA.tile([128, SG], f32, tag="big", name="op")
                for p in range(NPAIR):
                    nc.tensor.matmul(
                        op,
                        lhsT=ctxn[:, p, srow : srow + 128],
                        rhs=wo_sb[:, p, n * SG : (n + 1) * SG],
                        start=(p == 0),
                        stop=(p == NPAIR - 1),
                    )
                ob = out_pool.tile([128, SG], bf16, tag="ob", name="ob")
                if tail and out_q[0] % 2 == 0:
                    nc.scalar.copy(ob, op)
                else:
                    nc.vector.tensor_copy(ob, op)
                out_q[0] += 1
                nc.sync.dma_start(
                    out=out[srow : srow + 128, n * SG : (n + 1) * SG], in_=ob
                )

            def emit_attention(g, fillers):
                """Attention for q-group g, draining `fillers` into the
                ACT-bound inner loop. Returns the deferred normalize-finish
                closure of the last pair."""
                gs = slice(g * SG, (g + 1) * SG)
                nkc = (4 * g + 4) if causal else NKT
                slots = [2 * nkc]
                fin = None
                for p in range(NPAIR):
                    ctx2 = [
                        psB.tile([65, SG], f32, tag="ctx", name="ctx")
                        for _ in range(2)
                    ]

                    def emit_pv(kc, off, pt2, ctx2=ctx2, p=p, nkc=nkc):
                        for h_s in range(2):
                            nc.tensor.matmul(
                                ctx2[h_s][:, off:SG],
                                lhsT=vN[:, p, kc, h_s, :],
                                rhs=pt2[:, h_s, off:],
                                start=(kc == 0),
                                stop=(kc == nkc - 1),
                            )

                    # software-pipelined: sc/exp(kc) emitted before pv(kc-1)
                    prev = None
                    for kc in range(nkc):
                        tp_i = kc - 4 * g
                        diag = causal and tp_i >= 0
                        off = tp_i * 128 if diag else 0
                        sc2 = psA.tile([128, 2, SG], f32, tag="big", name="sc2")
                        for h_s in range(2):
                            hp = slice(h_s * 64, (h_s + 1) * 64)
                            nc.tensor.matmul(
                                sc2[:, h_s, off:],
                                lhsT=kT[hp, p, kc * 128 : (kc + 1) * 128],
                                rhs=qT[hp, p, g * SG + off : (g + 1) * SG],
                                start=True,
                                stop=True,
                            )
                        pt2 = pt_pool.tile([128, 2, SG], bf16, tag="pt", name="pt2")
                        nc.scalar.activation(
                            pt2[:, :, off:], sc2[:, :, off:], EXP, scale=0.125
                        )
                        if diag:
                            d0 = tp_i * 128
                            for h_s in range(2):
                                nc.vector.tensor_mul(
                                    pt2[:, h_s, d0 : d0 + 128],
                                    pt2[:, h_s, d0 : d0 + 128],
                                    mask,
                                )
                        # drain fillers: keep pace with remaining slots
                        pops = 1
                        if len(fillers) > slots[0] > 0:
                            pops = min(2, -(-len(fillers) // slots[0]))
                        for _ in range(pops):
                            if fillers:
                                fillers.pop(0)()
                        slots[0] -= 1
                        if prev is not None:
                            emit_pv(*prev)
                        prev = (kc, off, pt2)
                    emit_pv(*prev)

                    # ---- normalize phase 1: evict unnormalized ctx to SBUF
                    # (frees the psB banks; scalar engine does the PSUM-shift
                    # copies) and stage denominators at rows 0/32 for a
                    # lane-aligned reciprocal. Deferred into the NEXT pair's
                    # filler stream so it never sits between two pairs' exps
                    # in the ACT queue.
                    ctxu = cu_pool.tile([128, SG], f32, tag="cu", name="cu")
                    stg, stgr, stgb = stages[p]

                    def ph1_fn(p=p, ctx2=ctx2, ctxu=ctxu, stg=stg, stgr=stgr,
                               stgb=stgb):
                        for h_s in range(2):
                            nc.scalar.copy(
                                ctxu[h_s * 64 : (h_s + 1) * 64, :],
                                ctx2[h_s][0:64, :],
                            )
                            nc.vector.tensor_copy(
                                stg[32 * h_s : 32 * h_s + 1, :],
                                ctx2[h_s][64:65, :],
                            )
                        nc.vector.reciprocal_approx_fast(out=stgr, in_=stg)
                        nc.vector.tensor_copy(stgb, stgr)

                    def fin_fn(p=p, stgb=stgb, ctxu=ctxu):
                        # rank-1 broadcast of 1/den (PE) + per-head multiply
                        rb = psA.tile([128, SG], f32, tag="big", name="rb")
                        nc.tensor.matmul(
                            rb, lhsT=obk_sb, rhs=stgb, start=True, stop=True
                        )
                        rbs = rb_pool.tile([128, SG], f32, tag="rbs", name="rbs")
                        nc.vector.tensor_copy(rbs, rb)
                        for h_s in range(2):
                            hp = slice(h_s * 64, (h_s + 1) * 64)
                            nc.vector.tensor_mul(
                                ctxn[hp, p, gs], ctxu[hp, :], rbs[hp, :]
                            )

                    if p == 0:
                        fillers.insert(0, ph1_fn)
                        fillers.insert(min(3, len(fillers)), fin_fn)
                    else:
                        fin = (ph1_fn, fin_fn)
                # drain leftover fillers
                while fillers:
                    fillers.pop(0)()
                return fin

            def outproj_fillers(g):
                return [
                    (lambda g=g, st4=st4, n=n: emit_outproj_unit(g, st4, n))
                    for st4 in range(4)
                    for n in range(D // SG)
                ]

            if causal:
                # prologue: Q/K proj for g0, first V-nat tile; the rest of
                # g0's V-nat units lead the g0 filler stream.
                for u in emit_qk_proj_units(0, 0):
                    u()
                for u in emit_qk_proj_units(1, 0):
                    u()
                emit_vnat_unit(0)
                fin_prev = None
                for g in range(NSG):
                    fillers = []
                    if g == 0:
                        fillers += [lambda kt=kt: emit_vnat_unit(kt) for kt in (1, 2, 3)]
                    if fin_prev is not None:
                        fillers.append(fin_prev[0])
                        fillers.append(fin_prev[1])
                    if g > 0:
                        ops = outproj_fillers(g - 1)
                        vts = [
                            (lambda kt=kt: emit_vnat_unit(kt))
                            for kt in range(4 * g + 4, 4 * g + 8)
                        ] if g + 1 < NSG else []
                        mixed = []
                        for i, f in enumerate(ops):
                            mixed.append(f)
                            if i % 2 == 1 and vts:
                                mixed.append(vts.pop(0))
                        fillers += mixed + vts
                    elif g + 1 < NSG:
                        fillers += [
                            (lambda kt=kt: emit_vnat_unit(kt))
                            for kt in range(4 * g + 4, 4 * g + 8)
                        ]
                    if g + 1 < NSG:
                        fillers += emit_qk_proj_units(0, g + 1)
                        fillers += emit_qk_proj_units(1, g + 1)
                    fin_prev = emit_attention(g, fillers)
                fin_prev[0]()
                fin_prev[1]()
                for st4 in range(4):
                    for n in range(D // SG):
                        emit_outproj_unit(NSG - 1, st4, n, tail=True)
                if debug_dumps:
                    nc.sync.dma_start(out=d_qT, in_=qT[:, :, :])
                    nc.sync.dma_start(out=d_kT, in_=kT[:, :, :])
                    nc.sync.dma_start(out=d_vN, in_=vN[:, :, :, :, :])
                    nc.sync.dma_start(out=d_ctxn, in_=ctxn[:, :, :])
            else:
                # non-causal: attention needs ALL key tiles -> run every
                # projection first, then attention with outproj fillers.
                for g in range(NSG):
                    for u in emit_qk_proj_units(0, g):
                        u()
                    for u in emit_qk_proj_units(1, g):
                        u()
                    for kt in range(4 * g, 4 * g + 4):
                        emit_vnat_unit(kt)
                fin_prev = None
                for g in range(NSG):
                    fillers = []
                    if fin_prev is not None:
                        fillers.append(fin_prev[0])
                        fillers.append(fin_prev[1])
                    if g > 0:
                        fillers += outproj_fillers(g - 1)
                    fin_prev = emit_attention(g, fillers)
                fin_prev[0]()
                fin_prev[1]()
                for st4 in range(4):
                    for n in range(D // SG):
                        emit_outproj_unit(NSG - 1, st4, n, tail=True)

    nc.compile()
    return nc


def _core_inputs(query, key, value, Wq, bq, Wk, bk, Wv, bv, Wo, core):
    import ml_dtypes

    bf16 = ml_dtypes.bfloat16
    b = core // (NCORES // B)
    h0 = (core % (NCORES // B)) * HPC
    f32 = np.float32

    def packx(x):
        # [S, D] -> [128, NSG, NDC, SG]: xT[c*128+q, g*SG+s] at [q, g, c, s]
        xt = np.asarray(x, f32).T.reshape(NDC, 128, NSG, SG)
        return np.ascontiguousarray(xt.transpose(1, 2, 0, 3).astype(bf16))

    def packw(W):
        # [H, D, DH] -> per-pair [128, NDC, 128]
        outw = []
        for p in range(NPAIR):
            wp = np.concatenate([W[h0 + 2 * p], W[h0 + 2 * p + 1]], axis=1)  # [D,128]
            outw.append(wp.reshape(NDC, 128, 128).transpose(1, 0, 2))
        return np.ascontiguousarray(np.stack(outw).astype(bf16))

    def packb(bias):
        return np.ascontiguousarray(
            np.stack(
                [
                    np.concatenate([bias[h0 + 2 * p], bias[h0 + 2 * p + 1]])
                    for p in range(NPAIR)
                ]
            ).reshape(NPAIR, 128, 1),
            dtype=f32,
        )

    wo_p = np.ascontiguousarray(
        np.stack(
            [Wo[(h0 + 2 * p) * DH : (h0 + 2 * p + 2) * DH] for p in range(NPAIR)]
        ).astype(bf16)
    )
    wv_p = np.concatenate(
        [
            np.concatenate([Wv[h0 + 2 * p], Wv[h0 + 2 * p + 1]], axis=1)
            for p in range(NPAIR)
        ],
        axis=1,
    )  # [D, 256]
    wv_p = np.ascontiguousarray(
        wv_p.reshape(NDC, 128, 256).transpose(1, 0, 2).astype(bf16)
    )
    kk, qq = np.meshgrid(np.arange(128), np.arange(128), indexing="ij")
    mkk = (kk <= qq).astype(bf16)  # key <= query (keys on partitions)
    obk = np.zeros((33, 128), bf16)
    obk[0, 0:64] = 1.0
    obk[32, 64:128] = 1.0
    bvr = np.concatenate(
        [
            np.concatenate([bv[h0 + 2 * p], bv[h0 + 2 * p + 1]])
            for p in range(NPAIR)
        ]
    ).reshape(1, 256).astype(bf16)
    return {
        "mk": mkk,
        "on": np.ones((128, 64), bf16),
        "on1": np.ones((1, 128), bf16),
        "obk": obk,
        "xq": packx(query[b]),
        "xk": packx(key[b]),
        "xv": packx(value[b]),
        "wq": packw(Wq),
        "wk": packw(Wk),
        "wv": wv_p,
        "wo": wo_p,
        "bqk": np.stack([packb(bq), packb(bk)]),
        "bvr": bvr,
    }


LAST_RESULTS = None


def kernel(query, key, value, Wq, bq, Wk, bk, Wv, bv, Wo, bo, look_ahead_mask):
    global LAST_RESULTS
    from concourse.bass_utils import run_bass_kernel_spmd

    query = np.asarray(query, dtype=np.float32)
    key = np.asarray(key, dtype=np.float32)
    value = np.asarray(value, dtype=np.float32)
    Wq, Wk, Wv = (np.asarray(a, dtype=np.float32) for a in (Wq, Wk, Wv))
    bq, bk, bv = (np.asarray(a, dtype=np.float32) for a in (bq, bk, bv))
    Wo = np.asarray(Wo, dtype=np.float32)
    bo = np.asarray(bo, dtype=np.float32)
    causal = bool(np.asarray(look_ahead_mask).item())
    bz = not (np.any(bq) or np.any(bk) or np.any(bv))

    if (causal, bz) not in _BUILD_CACHE:
        _BUILD_CACHE[causal, bz] = _build(causal, bz)
    nc = _BUILD_CACHE[causal, bz]

    in_maps = [
        _core_inputs(query, key, value, Wq, bq, Wk, bk, Wv, bv, Wo, c)
        for c in range(NCORES)
    ]
    res = run_bass_kernel_spmd(nc, in_maps, core_ids=list(range(NCORES)))
    LAST_RESULTS = res

    gpb = NCORES // B
    out = np.stack(
        [
            np.sum(
                [
                    res.results[b * gpb + i]["out"].astype(np.float32)
                    for i in range(gpb)
                ],
                axis=0,
            )
            for b in range(B)
        ]
    )
    return (out + bo[None, None, :]).astype(np.float32)
